# revision 50
# baseline (speedup 1.0000x reference)
"""v5: SNN on 8 trn2 cores, data-parallel over batch.

Per core: B_c = 125,000 = NBL(25 lanes) x NCOLS(5000 cols); no padding
(8 * 125,000 = 1,000,000 exactly).

Precision scheme (emulated rel err 2.2e-3 vs 2e-2 budget):
  x = xh(fp16) + xl(fp8e5m2, exactly x - xh to ~2^-3 of the residual)
  L1 psum = w1h16.xh + w1l16.xh + w8e5.xl + ones*(-1/2) + r1.s1_{t-1}
  mem1 = beta*mem1 + psum1 (DVE fp32), s1 = Sign(mem1 - 1) in {-1,+1} fp16
  L2 psum = w2h16.s1 + w2l16.s1 + ones*(sum w2/2 - 1/2) + r2.s2_{t-1}
  mem2 = beta*mem2 + psum2 (fp32) -> out as fp16 (SWDGE cast-DMA)
  s2 = Sign(mem2 - 1) fp8e4 -> out directly (host maps {-1,+1}->{0,1})

IO per core: in 10MB fp16 + 5MB fp8; out 7.5MB fp16 + 3.75MB fp8.
DMA: one transfer per tensor per step (x on SP ring, spk on SP,
mem cast on gpsimd SWDGE). State tiles are full-width; Tile subtile
deps keep the 5 column-group pipelines independent.
"""

import numpy as np
from contextlib import ExitStack
from concurrent.futures import ThreadPoolExecutor

T = 10
NI, NH, NO = 4, 5, 3
BETA = 0.95
B_FULL = 1_000_000
NCORES = 8

NBL = 25
NCOLS = 5000
NPB = 1024
BC = NBL * NCOLS          # 125,000 per core

XR = NBL * NI             # 100
M1 = NBL * NH             # 125
M2 = NBL * NO             # 75


def set_geometry(ncols, npb=1024):
    global NCOLS, NPB, BC
    NCOLS, NPB = ncols, npb
    BC = NBL * NCOLS


def make_weights(w1, w2):
    import ml_dtypes
    e5 = ml_dtypes.float8_e5m2
    e4 = ml_dtypes.float8_e4m3
    f16 = np.float16
    w1 = np.asarray(w1, np.float64)
    w2 = np.asarray(w2, np.float64)
    # W1 [101, 125]: row (bl,i) -> col (bl,h); ones row 100 = -1/2
    w1f = np.zeros((XR + 1, M1), np.float64)
    for bl in range(NBL):
        w1f[4 * bl : 4 * bl + 4, 5 * bl : 5 * bl + 5] = w1.T
    w1f[XR, :] = -0.5
    w1h = w1f.astype(f16)
    w1l = (w1f - w1h.astype(np.float64)).astype(f16)
    w8 = w1f[0:XR].astype(e5)
    # r1 [125, 125] = -I/2 (fp16 exact)
    r1 = (-0.5 * np.eye(M1)).astype(f16)
    # W2 [126, 75]: row (bl,h) -> col (bl,o); ones row = sum w2/2 - 1/2
    w2f = np.zeros((M1 + 1, M2), np.float64)
    a = w2 / 2.0
    for bl in range(NBL):
        w2f[5 * bl : 5 * bl + 5, 3 * bl : 3 * bl + 3] = a.T
        w2f[M1, 3 * bl : 3 * bl + 3] = a.sum(axis=1) - 0.5
    w2h = w2f.astype(f16)
    w2l = (w2f - w2h.astype(np.float64)).astype(f16)
    # t=0 variant: const row without the -1/2 reset term (s2=-1 cancels it)
    w2f0 = w2f.copy()
    w2f0[M1] += 0.5
    w2h0 = w2f0.astype(f16)
    w2l0 = (w2f0 - w2h0.astype(np.float64)).astype(f16)
    # r2 [75, 75] = -I/2 (fp8e4 exact)
    r2 = (-0.5 * np.eye(M2)).astype(e4)
    return w1h, w1l, w8, r1, w2h, w2l, w2h0, w2l0, r2


def _split_multi_waits(nc):
    """Walrus codegen accepts only ONE sync-wait per compute instruction.
    Hoist extras onto pure-sync EventSemaphore instructions."""
    import concourse.mybir as mybir

    for f in nc.m.functions:
        for blk in f.blocks:
            out = []
            for ins in blk.instructions:
                si = ins.sync_info
                if (
                    si is not None
                    and len(si.on_wait) > 1
                    and not isinstance(ins, mybir.InstEventSemaphore)
                ):
                    waits = list(si.on_wait)
                    for j, w in enumerate(waits[:-1]):
                        out.append(
                            mybir.InstEventSemaphore(
                                name=f"{ins.name}-ws{j}",
                                engine=ins.engine,
                                ins=[],
                                outs=[],
                                sync_info=mybir.SyncInfo(
                                    on_wait=[w], on_update=[]
                                ),
                            )
                        )
                    ins.sync_info = mybir.SyncInfo(
                        on_wait=[waits[-1]], on_update=list(si.on_update)
                    )
                out.append(ins)
            blk.instructions = out


def build_nc_v5(split_waits=True, repeat=1, skip_in=False, skip_out=False,
                out_mode="sp", npb=None, ps_bufs=2, skip_act=False,
                skip_dve=False, skew=1, stt2_pool=0, fine1=False):
    import concourse.bass as bass
    import concourse.mybir as mybir
    from concourse.tile import TileContext

    f32 = mybir.dt.float32
    f16 = mybir.dt.float16
    e5 = mybir.dt.float8e5
    e4 = mybir.dt.float8e4
    Act = mybir.ActivationFunctionType
    mult = mybir.AluOpType.mult
    add = mybir.AluOpType.add

    groups = []
    c0 = 0
    while c0 < NCOLS:
        n = min(npb or NPB, NCOLS - c0)
        groups.append((c0, n))
        c0 += n

    nc = bass.Bass()
    xh_d = nc.declare_dram_parameter("xh", [T, XR, NCOLS], f16, isOutput=False)
    xl_d = nc.declare_dram_parameter("xl", [T, XR, NCOLS], e5, isOutput=False)
    w1h_d = nc.declare_dram_parameter("w1h", [XR + 1, M1], f16, isOutput=False)
    w1l_d = nc.declare_dram_parameter("w1l", [XR + 1, M1], f16, isOutput=False)
    w8_d = nc.declare_dram_parameter("w8", [XR, M1], e5, isOutput=False)
    r1_d = nc.declare_dram_parameter("r1", [M1, M1], f16, isOutput=False)
    w2h_d = nc.declare_dram_parameter("w2h", [M1 + 1, M2], f16, isOutput=False)
    w2l_d = nc.declare_dram_parameter("w2l", [M1 + 1, M2], f16, isOutput=False)
    w2h0_d = nc.declare_dram_parameter("w2h0", [M1 + 1, M2], f16, isOutput=False)
    w2l0_d = nc.declare_dram_parameter("w2l0", [M1 + 1, M2], f16, isOutput=False)
    r2_d = nc.declare_dram_parameter("r2", [M2, M2], e4, isOutput=False)
    ones_d = nc.declare_dram_parameter("ones", [1, NCOLS], f16, isOutput=False)
    spk_d = nc.declare_dram_parameter("spk2", [T, M2, NCOLS], e4, isOutput=True)
    mem_d = nc.declare_dram_parameter("mem2", [T, M2, NCOLS], f32, isOutput=True)

    with ExitStack() as ctx:
        tc = ctx.enter_context(TileContext(nc))
        wp = ctx.enter_context(tc.tile_pool(name="wp", bufs=1))
        st = ctx.enter_context(tc.tile_pool(name="st", bufs=1))
        xp = ctx.enter_context(tc.tile_pool(name="xp", bufs=1))
        ps = ctx.enter_context(tc.tile_pool(name="ps", bufs=ps_bufs, space="PSUM"))

        negone = wp.tile([128, 1], f32, tag="negone")
        nc.vector.memset(negone[:], -1.0)
        w1h = wp.tile([XR + 1, M1], f16, tag="w1h")
        w1l = wp.tile([XR + 1, M1], f16, tag="w1l")
        w8 = wp.tile([XR, M1], e5, tag="w8")
        r1 = wp.tile([M1, M1], f16, tag="r1")
        w2h = wp.tile([M1 + 1, M2], f16, tag="w2h")
        w2l = wp.tile([M1 + 1, M2], f16, tag="w2l")
        w2h0 = wp.tile([M1 + 1, M2], f16, tag="w2h0")
        w2l0 = wp.tile([M1 + 1, M2], f16, tag="w2l0")
        r2 = wp.tile([M2, M2], e4, tag="r2")
        for tl, dr in ((w1h, w1h_d), (w1l, w1l_d), (w8, w8_d), (r1, r1_d),
                       (w2h, w2h_d), (w2l, w2l_d), (w2h0, w2h0_d),
                       (w2l0, w2l0_d), (r2, r2_d)):
            nc.sync.dma_start(tl[:], dr[:])

        # persistent state (full width; subtile deps pipeline the groups)
        m1t = st.tile([M1, NCOLS], f32, tag="m1t", name="m1t")
        s1t = st.tile([M1 + 1, NCOLS], f16, tag="s1t", name="s1t")
        # layer-2 state ping-pongs by step parity: the output DMA reads one
        # buffer while the next step's update writes the other (kills the
        # per-step WAR stall on DMA completion)
        m2t_pp = [st.tile([M2, NCOLS], f32, tag=f"m2t{p}", name=f"m2t{p}")
                  for p in range(2)]
        s2t_pp = [st.tile([M2, NCOLS], e4, tag=f"s2t{p}", name=f"s2t{p}")
                  for p in range(2)]
        # no state memsets: the t=0 path writes every state tile before any
        # read (constants fold exactly: ones*(-1/2) + r.(-1) == 0)
        nc.sync.dma_start(s1t[M1 : M1 + 1, :], ones_d[:])
        if skip_act:   # timing-ablation only: states init-once
            nc.vector.memset(s1t[0:M1, :], -1.0)
            for p in range(2):
                nc.vector.memset(s2t_pp[p][:], -1.0)
        if skip_dve:
            nc.vector.memset(m1t[:], 0.0)
            for p in range(2):
                nc.vector.memset(m2t_pp[p][:], 0.0)

        # x ring buffers (3-deep): ones row preset once
        xhs = [xp.tile([XR + 1, NCOLS], f16, tag=f"xh_{r}", name=f"xh_{r}")
               for r in range(3)]
        xls = [xp.tile([XR, NCOLS], e5, tag=f"xl_{r}", name=f"xl_{r}")
               for r in range(3)]
        for r in range(3):
            nc.sync.dma_start(xhs[r][XR : XR + 1, :], ones_d[:])
            if skip_in:
                nc.sync.dma_start(xhs[r][0:XR, :], xh_d[r])
                nc.sync.dma_start(xls[r][:, :], xl_d[r])

        def mm(out_ap, w_ap, rhs_ap, start, stop):
            n = out_ap.shape[-1]
            o = 0
            while o < n:
                k = min(512, n - o)
                nc.tensor.matmul(
                    out_ap[:, o : o + k], w_ap, rhs_ap[:, o : o + k],
                    start=start, stop=stop,
                )
                o += k

        for tg in range(repeat * T):
            t = tg % T
            ring = tg % 3
            xh_, xl_ = xhs[ring], xls[ring]
            if not skip_in:
                if tg == 0 and len(groups) > 1:
                    # cold start: land the first group's columns first so
                    # the first matmul isn't gated on the full transfer
                    n0 = groups[0][1]
                    nc.sync.dma_start(xh_[0:XR, 0:n0], xh_d[t, :, 0:n0])
                    nc.sync.dma_start(xl_[:, 0:n0], xl_d[t, :, 0:n0])
                    nc.sync.dma_start(xh_[0:XR, n0:NCOLS], xh_d[t, :, n0:NCOLS])
                    nc.sync.dma_start(xl_[:, n0:NCOLS], xl_d[t, :, n0:NCOLS])
                else:
                    nc.sync.dma_start(xh_[0:XR, :], xh_d[t])
                    nc.sync.dma_start(xl_[:, :], xl_d[t])
            first = tg == 0

            def emit_L1(g, tg=tg, xh_=xh_, xl_=xl_, first=first):
                c0, n = groups[g]
                cs = slice(c0, c0 + n)
                ps1 = ps.tile([M1, n], f32, tag="ps1", name=f"ps1_{tg}_{c0}")
                if first:
                    mm(ps1[:, 0:n], w1h[0:XR, :], xh_[0:XR, cs],
                       start=True, stop=False)
                    mm(ps1[:, 0:n], w1l[0:XR, :], xh_[0:XR, cs],
                       start=False, stop=False)
                    mm(ps1[:, 0:n], w8[:], xl_[:, cs], start=False, stop=True)
                    nc.vector.tensor_scalar_add(m1t[:, cs], ps1[:, 0:n], 0.0)
                else:
                    mm(ps1[:, 0:n], w1h[:], xh_[:, cs], start=True, stop=False)
                    mm(ps1[:, 0:n], w1l[:], xh_[:, cs], start=False, stop=False)
                    mm(ps1[:, 0:n], w8[:], xl_[:, cs], start=False, stop=False)
                    mm(ps1[:, 0:n], r1[:], s1t[0:M1, cs],
                       start=False, stop=True)
                    if not skip_dve:
                        if fine1 and n > 512:
                            for o in range(0, n, 512):
                                k = min(512, n - o)
                                os_ = slice(c0 + o, c0 + o + k)
                                nc.vector.scalar_tensor_tensor(
                                    m1t[:, os_], m1t[:, os_], BETA,
                                    ps1[:, o : o + k], mult, add,
                                )
                                nc.scalar.activation(
                                    s1t[0:M1, os_], m1t[:, os_],
                                    Act.Sign, bias=negone[0:M1, :],
                                )
                        else:
                            nc.vector.scalar_tensor_tensor(
                                m1t[:, cs], m1t[:, cs], BETA, ps1[:, 0:n],
                                mult, add,
                            )
                if not skip_act and not (fine1 and n > 512 and not first
                                         and not skip_dve):
                    nc.scalar.activation(
                        s1t[0:M1, cs], ps1[:, 0:n] if skip_dve else m1t[:, cs],
                        Act.Sign, bias=negone[0:M1, :],
                    )
                elif skip_dve:
                    nc.scalar.activation(
                        s1t[0:M1, cs], ps1[:, 0:n], Act.Sign,
                        bias=negone[0:M1, :],
                    )

            def emit_L2(g, tg=tg, first=first):
                c0, n = groups[g]
                cs = slice(c0, c0 + n)
                cur, prv = tg % 2, (tg - 1) % 2
                m2c, m2p = m2t_pp[cur], m2t_pp[prv]
                s2c, s2p = s2t_pp[cur], s2t_pp[prv]
                ps2 = ps.tile([M2, n], f32, tag="ps2", name=f"ps2_{tg}_{c0}")
                if first:
                    mm(ps2[:, 0:n], w2h0[:], s1t[:, cs], start=True, stop=False)
                    mm(ps2[:, 0:n], w2l0[:], s1t[:, cs], start=False, stop=True)
                    nc.vector.tensor_scalar_add(m2c[:, cs], ps2[:, 0:n], 0.0)
                else:
                    # r2 first: its operand (s2 of step t-1) is ready before
                    # this step's s1, so PE needn't wait on ACT
                    mm(ps2[:, 0:n], r2[:], s2p[:, cs], start=True, stop=False)
                    mm(ps2[:, 0:n], w2h[:], s1t[:, cs], start=False, stop=False)
                    mm(ps2[:, 0:n], w2l[:], s1t[:, cs], start=False, stop=True)
                    if not skip_dve:
                        use_pool = (stt2_pool == 2
                                    or (stt2_pool == 1 and g % 2 == 1))
                        eng = nc.gpsimd if use_pool else nc.vector
                        eng.scalar_tensor_tensor(
                            m2c[:, cs], m2p[:, cs], BETA, ps2[:, 0:n],
                            mult, add,
                        )
                if not skip_act:
                    nc.scalar.activation(
                        s2c[:, cs], ps2[:, 0:n] if skip_dve else m2c[:, cs],
                        Act.Sign, bias=negone[0:M2, :],
                    )
                elif skip_dve:
                    nc.scalar.activation(
                        s2c[:, cs], ps2[:, 0:n], Act.Sign,
                        bias=negone[0:M2, :],
                    )

            # software-pipelined emission: engines execute their streams in
            # FIFO order, so L2(g) (gated on this group's ACT sign) is
            # emitted `skew` groups behind L1 to avoid head-of-line blocking
            ng = len(groups)
            for slot in range(ng + skew):
                if slot < ng:
                    emit_L1(slot)
                if slot >= skew:
                    emit_L2(slot - skew)
            if not skip_out:
                m2c, s2c = m2t_pp[tg % 2], s2t_pp[tg % 2]
                if out_mode == "sp":
                    nc.sync.dma_start(mem_d[t], m2c[:, :])
                    nc.sync.dma_start(spk_d[t], s2c[:, :])
                else:  # "act"
                    nc.scalar.dma_start(mem_d[t], m2c[:, :])
                    nc.scalar.dma_start(spk_d[t], s2c[:, :])

    if split_waits:
        _split_multi_waits(nc)
    return nc


def prep_core_x(xh_full, xl_full, c):
    # [T, BC, NI] -> rows (bl, i) x cols
    sl = slice(c * BC, (c + 1) * BC)
    out = []
    for arr in (xh_full, xl_full):
        v = arr[:, sl, :].reshape(T, NBL, NCOLS, NI)
        v = np.ascontiguousarray(v.transpose(0, 1, 3, 2)).reshape(T, XR, NCOLS)
        out.append(v)
    return out


def unpack_core(res_c, spk2, mem2, c):
    sl = slice(c * BC, (c + 1) * BC)
    s = res_c["spk2"]   # [T, 75, NCOLS] fp8e4 in {-1,+1}
    m = res_c["mem2"]   # [T, 75, NCOLS] fp16
    sv = s.astype(np.float32).reshape(T, NBL, NO, NCOLS).transpose(0, 1, 3, 2)
    mv = m.astype(np.float32).reshape(T, NBL, NO, NCOLS).transpose(0, 1, 3, 2)
    spk2[:, sl, :] = ((sv + 1.0) * 0.5).reshape(T, BC, NO)
    mem2[:, sl, :] = mv.reshape(T, BC, NO)


def prepare_in_maps(x, w1, w2):
    import ml_dtypes
    w1h, w1l, w8, r1, w2h, w2l, w2h0, w2l0, r2 = make_weights(w1, w2)
    xh_full = x.astype(np.float16)
    xl_full = (x - xh_full.astype(np.float32)).astype(ml_dtypes.float8_e5m2)
    with ThreadPoolExecutor(8) as ex:
        xs = list(ex.map(lambda c: prep_core_x(xh_full, xl_full, c),
                         range(NCORES)))
    onesv = np.ones((1, NCOLS), np.float16)
    return [
        {"xh": xs[c][0], "xl": xs[c][1], "w1h": w1h, "w1l": w1l, "w8": w8,
         "r1": r1, "w2h": w2h, "w2l": w2l, "w2h0": w2h0, "w2l0": w2l0,
         "r2": r2, "ones": onesv}
        for c in range(NCORES)
    ]


def kernel(**inputs):
    x = np.asarray(inputs["x"], dtype=np.float32)
    w1 = np.asarray(inputs["w1"], dtype=np.float32)
    w2 = np.asarray(inputs["w2"], dtype=np.float32)

    from concourse.bass_utils import run_bass_kernel_spmd

    nc = build_nc_v5()
    in_maps = prepare_in_maps(x, w1, w2)

    import time as _time
    _t0 = _time.time()
    res = run_bass_kernel_spmd(nc, in_maps, list(range(NCORES))).results
    print(f"[kernel5] device compile+run {_time.time()-_t0:.1f}s", flush=True)

    spk2 = np.empty((T, B_FULL, NO), dtype=np.float32)
    mem2 = np.empty((T, B_FULL, NO), dtype=np.float32)
    with ThreadPoolExecutor(8) as ex:
        list(ex.map(lambda c: unpack_core(res[c], spk2, mem2, c),
                    range(NCORES)))
    return spk2, mem2


# revision 53
# speedup vs baseline: 1.0688x; 1.0688x over previous
"""v5: SNN on 8 trn2 cores, data-parallel over batch.

Per core: B_c = 125,000 = NBL(25 lanes) x NCOLS(5000 cols); no padding
(8 * 125,000 = 1,000,000 exactly).

Precision scheme (emulated rel err 2.2e-3 vs 2e-2 budget):
  x = xh(fp16) + xl(fp8e5m2, exactly x - xh to ~2^-3 of the residual)
  L1 psum = w1h16.xh + w1l16.xh + w8e5.xl + ones*(-1/2) + r1.s1_{t-1}
  mem1 = beta*mem1 + psum1 (DVE fp32), s1 = Sign(mem1 - 1) in {-1,+1} fp16
  L2 psum = w2h16.s1 + w2l16.s1 + ones*(sum w2/2 - 1/2) + r2.s2_{t-1}
  mem2 = beta*mem2 + psum2 (fp32) -> out as fp16 (SWDGE cast-DMA)
  s2 = Sign(mem2 - 1) fp8e4 -> out directly (host maps {-1,+1}->{0,1})

IO per core: in 10MB fp16 + 5MB fp8; out 7.5MB fp16 + 3.75MB fp8.
DMA: one transfer per tensor per step (x on SP ring, spk on SP,
mem cast on gpsimd SWDGE). State tiles are full-width; Tile subtile
deps keep the 5 column-group pipelines independent.
"""

import numpy as np
from contextlib import ExitStack
from concurrent.futures import ThreadPoolExecutor

T = 10
NI, NH, NO = 4, 5, 3
BETA = 0.95
B_FULL = 1_000_000
NCORES = 8

NBL = 25
NCOLS = 5000
NPB = 1024
BC = NBL * NCOLS          # 125,000 per core

XR = NBL * NI             # 100
M1 = NBL * NH             # 125
M2 = NBL * NO             # 75


def set_geometry(ncols, npb=1024):
    global NCOLS, NPB, BC
    NCOLS, NPB = ncols, npb
    BC = NBL * NCOLS


def make_weights(w1, w2):
    import ml_dtypes
    e5 = ml_dtypes.float8_e5m2
    e4 = ml_dtypes.float8_e4m3
    f16 = np.float16
    w1 = np.asarray(w1, np.float64)
    w2 = np.asarray(w2, np.float64)
    # all lhsT free dims padded with zero columns to 128 so the compiler's
    # Fast-Weight-Load kicks in (requires NumWeights==128); the extra psum
    # rows compute zeros and are never read
    # W1 [101, 128]: row (bl,i) -> col (bl,h); ones row 100 = -1/2
    w1f = np.zeros((XR + 1, 128), np.float64)
    for bl in range(NBL):
        w1f[4 * bl : 4 * bl + 4, 5 * bl : 5 * bl + 5] = w1.T
    w1f[XR, 0:M1] = -0.5
    w1h = w1f.astype(f16)
    w1l = (w1f - w1h.astype(np.float64)).astype(f16)
    w8 = w1f[0:XR].astype(e5)
    # r1 [125, 128] = -I/2 (fp16 exact)
    r1 = np.zeros((M1, 128), np.float64)
    r1[:, 0:M1] = -0.5 * np.eye(M1)
    r1 = r1.astype(f16)
    # W2 [126, 128]: row (bl,h) -> col (bl,o); ones row = sum w2/2 - 1/2
    w2f = np.zeros((M1 + 1, 128), np.float64)
    a = w2 / 2.0
    for bl in range(NBL):
        w2f[5 * bl : 5 * bl + 5, 3 * bl : 3 * bl + 3] = a.T
        w2f[M1, 3 * bl : 3 * bl + 3] = a.sum(axis=1) - 0.5
    w2h = w2f.astype(f16)
    w2l = (w2f - w2h.astype(np.float64)).astype(f16)
    # t=0 variant: const row without the -1/2 reset term (s2=-1 cancels it)
    w2f0 = w2f.copy()
    w2f0[M1, 0:M2] += 0.5
    w2h0 = w2f0.astype(f16)
    w2l0 = (w2f0 - w2h0.astype(np.float64)).astype(f16)
    # r2 [75, 128] = -I/2 (fp8e4 exact)
    r2 = np.zeros((M2, 128), np.float64)
    r2[:, 0:M2] = -0.5 * np.eye(M2)
    r2 = r2.astype(e4)
    return w1h, w1l, w8, r1, w2h, w2l, w2h0, w2l0, r2


def _split_multi_waits(nc):
    """Walrus codegen accepts only ONE sync-wait per compute instruction.
    Hoist extras onto pure-sync EventSemaphore instructions."""
    import concourse.mybir as mybir

    for f in nc.m.functions:
        for blk in f.blocks:
            out = []
            for ins in blk.instructions:
                si = ins.sync_info
                if (
                    si is not None
                    and len(si.on_wait) > 1
                    and not isinstance(ins, mybir.InstEventSemaphore)
                ):
                    waits = list(si.on_wait)
                    for j, w in enumerate(waits[:-1]):
                        out.append(
                            mybir.InstEventSemaphore(
                                name=f"{ins.name}-ws{j}",
                                engine=ins.engine,
                                ins=[],
                                outs=[],
                                sync_info=mybir.SyncInfo(
                                    on_wait=[w], on_update=[]
                                ),
                            )
                        )
                    ins.sync_info = mybir.SyncInfo(
                        on_wait=[waits[-1]], on_update=list(si.on_update)
                    )
                out.append(ins)
            blk.instructions = out


def build_nc_v5(split_waits=True, repeat=1, skip_in=False, skip_out=False,
                out_mode="sp", npb=None, ps_bufs=2, skip_act=False,
                skip_dve=False, skew=1, stt2_pool=0, fine1=False):
    import concourse.bass as bass
    import concourse.mybir as mybir
    from concourse.tile import TileContext

    f32 = mybir.dt.float32
    f16 = mybir.dt.float16
    e5 = mybir.dt.float8e5
    e4 = mybir.dt.float8e4
    Act = mybir.ActivationFunctionType
    mult = mybir.AluOpType.mult
    add = mybir.AluOpType.add

    groups = []
    c0 = 0
    while c0 < NCOLS:
        n = min(npb or NPB, NCOLS - c0)
        groups.append((c0, n))
        c0 += n

    nc = bass.Bass()
    xh_d = nc.declare_dram_parameter("xh", [T, XR, NCOLS], f16, isOutput=False)
    xl_d = nc.declare_dram_parameter("xl", [T, XR, NCOLS], e5, isOutput=False)
    w1h_d = nc.declare_dram_parameter("w1h", [XR + 1, 128], f16, isOutput=False)
    w1l_d = nc.declare_dram_parameter("w1l", [XR + 1, 128], f16, isOutput=False)
    w8_d = nc.declare_dram_parameter("w8", [XR, 128], e5, isOutput=False)
    r1_d = nc.declare_dram_parameter("r1", [M1, 128], f16, isOutput=False)
    w2h_d = nc.declare_dram_parameter("w2h", [M1 + 1, 128], f16, isOutput=False)
    w2l_d = nc.declare_dram_parameter("w2l", [M1 + 1, 128], f16, isOutput=False)
    w2h0_d = nc.declare_dram_parameter("w2h0", [M1 + 1, 128], f16, isOutput=False)
    w2l0_d = nc.declare_dram_parameter("w2l0", [M1 + 1, 128], f16, isOutput=False)
    r2_d = nc.declare_dram_parameter("r2", [M2, 128], e4, isOutput=False)
    ones_d = nc.declare_dram_parameter("ones", [1, NCOLS], f16, isOutput=False)
    spk_d = nc.declare_dram_parameter("spk2", [T, M2, NCOLS], e4, isOutput=True)
    mem_d = nc.declare_dram_parameter("mem2", [T, M2, NCOLS], f32, isOutput=True)

    with ExitStack() as ctx:
        tc = ctx.enter_context(TileContext(nc))
        wp = ctx.enter_context(tc.tile_pool(name="wp", bufs=1))
        st = ctx.enter_context(tc.tile_pool(name="st", bufs=1))
        xp = ctx.enter_context(tc.tile_pool(name="xp", bufs=1))
        ps = ctx.enter_context(tc.tile_pool(name="ps", bufs=ps_bufs, space="PSUM"))

        negone = wp.tile([128, 1], f32, tag="negone")
        nc.vector.memset(negone[:], -1.0)
        w1h = wp.tile([XR + 1, 128], f16, tag="w1h")
        w1l = wp.tile([XR + 1, 128], f16, tag="w1l")
        w8 = wp.tile([XR, 128], e5, tag="w8")
        r1 = wp.tile([M1, 128], f16, tag="r1")
        w2h = wp.tile([M1 + 1, 128], f16, tag="w2h")
        w2l = wp.tile([M1 + 1, 128], f16, tag="w2l")
        w2h0 = wp.tile([M1 + 1, 128], f16, tag="w2h0")
        w2l0 = wp.tile([M1 + 1, 128], f16, tag="w2l0")
        r2 = wp.tile([M2, 128], e4, tag="r2")
        for tl, dr in ((w1h, w1h_d), (w1l, w1l_d), (w8, w8_d), (r1, r1_d),
                       (w2h, w2h_d), (w2l, w2l_d), (w2h0, w2h0_d),
                       (w2l0, w2l0_d), (r2, r2_d)):
            nc.sync.dma_start(tl[:], dr[:])

        # persistent state (full width; subtile deps pipeline the groups)
        m1t = st.tile([M1, NCOLS], f32, tag="m1t", name="m1t")
        s1t = st.tile([M1 + 1, NCOLS], f16, tag="s1t", name="s1t")
        # layer-2 state ping-pongs by step parity: the output DMA reads one
        # buffer while the next step's update writes the other (kills the
        # per-step WAR stall on DMA completion)
        m2t_pp = [st.tile([M2, NCOLS], f32, tag=f"m2t{p}", name=f"m2t{p}")
                  for p in range(2)]
        s2t_pp = [st.tile([M2, NCOLS], e4, tag=f"s2t{p}", name=f"s2t{p}")
                  for p in range(2)]
        # no state memsets: the t=0 path writes every state tile before any
        # read (constants fold exactly: ones*(-1/2) + r.(-1) == 0)
        nc.sync.dma_start(s1t[M1 : M1 + 1, :], ones_d[:])
        if skip_act:   # timing-ablation only: states init-once
            nc.vector.memset(s1t[0:M1, :], -1.0)
            for p in range(2):
                nc.vector.memset(s2t_pp[p][:], -1.0)
        if skip_dve:
            nc.vector.memset(m1t[:], 0.0)
            for p in range(2):
                nc.vector.memset(m2t_pp[p][:], 0.0)

        # x ring buffers (3-deep): ones row preset once
        xhs = [xp.tile([XR + 1, NCOLS], f16, tag=f"xh_{r}", name=f"xh_{r}")
               for r in range(3)]
        xls = [xp.tile([XR, NCOLS], e5, tag=f"xl_{r}", name=f"xl_{r}")
               for r in range(3)]
        for r in range(3):
            nc.sync.dma_start(xhs[r][XR : XR + 1, :], ones_d[:])
            if skip_in:
                nc.sync.dma_start(xhs[r][0:XR, :], xh_d[r])
                nc.sync.dma_start(xls[r][:, :], xl_d[r])

        def mm(out_ap, w_ap, rhs_ap, start, stop):
            n = out_ap.shape[-1]
            o = 0
            while o < n:
                k = min(512, n - o)
                nc.tensor.matmul(
                    out_ap[:, o : o + k], w_ap, rhs_ap[:, o : o + k],
                    start=start, stop=stop,
                )
                o += k

        for tg in range(repeat * T):
            t = tg % T
            ring = tg % 3
            xh_, xl_ = xhs[ring], xls[ring]
            if not skip_in:
                if tg == 0 and len(groups) > 1:
                    # cold start: land the first group's columns first so
                    # the first matmul isn't gated on the full transfer
                    n0 = groups[0][1]
                    nc.sync.dma_start(xh_[0:XR, 0:n0], xh_d[t, :, 0:n0])
                    nc.sync.dma_start(xl_[:, 0:n0], xl_d[t, :, 0:n0])
                    nc.sync.dma_start(xh_[0:XR, n0:NCOLS], xh_d[t, :, n0:NCOLS])
                    nc.sync.dma_start(xl_[:, n0:NCOLS], xl_d[t, :, n0:NCOLS])
                else:
                    nc.sync.dma_start(xh_[0:XR, :], xh_d[t])
                    nc.sync.dma_start(xl_[:, :], xl_d[t])
            first = tg == 0

            def emit_L1(g, tg=tg, xh_=xh_, xl_=xl_, first=first):
                c0, n = groups[g]
                cs = slice(c0, c0 + n)
                ps1 = ps.tile([128, n], f32, tag="ps1", name=f"ps1_{tg}_{c0}")
                if first:
                    mm(ps1[:, 0:n], w1h[0:XR, :], xh_[0:XR, cs],
                       start=True, stop=False)
                    mm(ps1[:, 0:n], w1l[0:XR, :], xh_[0:XR, cs],
                       start=False, stop=False)
                    mm(ps1[:, 0:n], w8[:], xl_[:, cs], start=False, stop=True)
                    nc.vector.tensor_scalar_add(m1t[:, cs], ps1[0:M1, 0:n], 0.0)
                else:
                    mm(ps1[:, 0:n], w1h[:], xh_[:, cs], start=True, stop=False)
                    mm(ps1[:, 0:n], w1l[:], xh_[:, cs], start=False, stop=False)
                    mm(ps1[:, 0:n], w8[:], xl_[:, cs], start=False, stop=False)
                    mm(ps1[:, 0:n], r1[:], s1t[0:M1, cs],
                       start=False, stop=True)
                    if not skip_dve:
                        if fine1 and n > 512:
                            for o in range(0, n, 512):
                                k = min(512, n - o)
                                os_ = slice(c0 + o, c0 + o + k)
                                nc.vector.scalar_tensor_tensor(
                                    m1t[:, os_], m1t[:, os_], BETA,
                                    ps1[0:M1, o : o + k], mult, add,
                                )
                                nc.scalar.activation(
                                    s1t[0:M1, os_], m1t[:, os_],
                                    Act.Sign, bias=negone[0:M1, :],
                                )
                        else:
                            nc.vector.scalar_tensor_tensor(
                                m1t[:, cs], m1t[:, cs], BETA, ps1[0:M1, 0:n],
                                mult, add,
                            )
                if not skip_act and not (fine1 and n > 512 and not first
                                         and not skip_dve):
                    nc.scalar.activation(
                        s1t[0:M1, cs], ps1[0:M1, 0:n] if skip_dve else m1t[:, cs],
                        Act.Sign, bias=negone[0:M1, :],
                    )
                elif skip_dve:
                    nc.scalar.activation(
                        s1t[0:M1, cs], ps1[0:M1, 0:n], Act.Sign,
                        bias=negone[0:M1, :],
                    )

            def emit_L2(g, tg=tg, first=first):
                c0, n = groups[g]
                cs = slice(c0, c0 + n)
                cur, prv = tg % 2, (tg - 1) % 2
                m2c, m2p = m2t_pp[cur], m2t_pp[prv]
                s2c, s2p = s2t_pp[cur], s2t_pp[prv]
                ps2 = ps.tile([128, n], f32, tag="ps2", name=f"ps2_{tg}_{c0}")
                if first:
                    mm(ps2[:, 0:n], w2h0[:], s1t[:, cs], start=True, stop=False)
                    mm(ps2[:, 0:n], w2l0[:], s1t[:, cs], start=False, stop=True)
                    nc.vector.tensor_scalar_add(m2c[:, cs], ps2[0:M2, 0:n], 0.0)
                else:
                    # r2 first: its operand (s2 of step t-1) is ready before
                    # this step's s1, so PE needn't wait on ACT
                    mm(ps2[:, 0:n], r2[:], s2p[:, cs], start=True, stop=False)
                    mm(ps2[:, 0:n], w2h[:], s1t[:, cs], start=False, stop=False)
                    mm(ps2[:, 0:n], w2l[:], s1t[:, cs], start=False, stop=True)
                    if not skip_dve:
                        use_pool = (stt2_pool == 2
                                    or (stt2_pool == 1 and g % 2 == 1))
                        eng = nc.gpsimd if use_pool else nc.vector
                        eng.scalar_tensor_tensor(
                            m2c[:, cs], m2p[:, cs], BETA, ps2[0:M2, 0:n],
                            mult, add,
                        )
                if not skip_act:
                    nc.scalar.activation(
                        s2c[:, cs], ps2[0:M2, 0:n] if skip_dve else m2c[:, cs],
                        Act.Sign, bias=negone[0:M2, :],
                    )
                elif skip_dve:
                    nc.scalar.activation(
                        s2c[:, cs], ps2[0:M2, 0:n], Act.Sign,
                        bias=negone[0:M2, :],
                    )

            # software-pipelined emission: engines execute their streams in
            # FIFO order, so L2(g) (gated on this group's ACT sign) is
            # emitted `skew` groups behind L1 to avoid head-of-line blocking
            ng = len(groups)
            for slot in range(ng + skew):
                if slot < ng:
                    emit_L1(slot)
                if slot >= skew:
                    emit_L2(slot - skew)
            if not skip_out:
                m2c, s2c = m2t_pp[tg % 2], s2t_pp[tg % 2]
                if out_mode == "sp":
                    nc.sync.dma_start(mem_d[t], m2c[:, :])
                    nc.sync.dma_start(spk_d[t], s2c[:, :])
                else:  # "act"
                    nc.scalar.dma_start(mem_d[t], m2c[:, :])
                    nc.scalar.dma_start(spk_d[t], s2c[:, :])

    if split_waits:
        _split_multi_waits(nc)
    return nc


def prep_core_x(xh_full, xl_full, c):
    # [T, BC, NI] -> rows (bl, i) x cols
    sl = slice(c * BC, (c + 1) * BC)
    out = []
    for arr in (xh_full, xl_full):
        v = arr[:, sl, :].reshape(T, NBL, NCOLS, NI)
        v = np.ascontiguousarray(v.transpose(0, 1, 3, 2)).reshape(T, XR, NCOLS)
        out.append(v)
    return out


def unpack_core(res_c, spk2, mem2, c):
    sl = slice(c * BC, (c + 1) * BC)
    s = res_c["spk2"]   # [T, 75, NCOLS] fp8e4 in {-1,+1}
    m = res_c["mem2"]   # [T, 75, NCOLS] fp16
    sv = s.astype(np.float32).reshape(T, NBL, NO, NCOLS).transpose(0, 1, 3, 2)
    mv = m.astype(np.float32).reshape(T, NBL, NO, NCOLS).transpose(0, 1, 3, 2)
    spk2[:, sl, :] = ((sv + 1.0) * 0.5).reshape(T, BC, NO)
    mem2[:, sl, :] = mv.reshape(T, BC, NO)


def prepare_in_maps(x, w1, w2):
    import ml_dtypes
    w1h, w1l, w8, r1, w2h, w2l, w2h0, w2l0, r2 = make_weights(w1, w2)
    xh_full = x.astype(np.float16)
    xl_full = (x - xh_full.astype(np.float32)).astype(ml_dtypes.float8_e5m2)
    with ThreadPoolExecutor(8) as ex:
        xs = list(ex.map(lambda c: prep_core_x(xh_full, xl_full, c),
                         range(NCORES)))
    onesv = np.ones((1, NCOLS), np.float16)
    return [
        {"xh": xs[c][0], "xl": xs[c][1], "w1h": w1h, "w1l": w1l, "w8": w8,
         "r1": r1, "w2h": w2h, "w2l": w2l, "w2h0": w2h0, "w2l0": w2l0,
         "r2": r2, "ones": onesv}
        for c in range(NCORES)
    ]


def kernel(**inputs):
    x = np.asarray(inputs["x"], dtype=np.float32)
    w1 = np.asarray(inputs["w1"], dtype=np.float32)
    w2 = np.asarray(inputs["w2"], dtype=np.float32)

    from concourse.bass_utils import run_bass_kernel_spmd

    nc = build_nc_v5()
    in_maps = prepare_in_maps(x, w1, w2)

    import time as _time
    _t0 = _time.time()
    res = run_bass_kernel_spmd(nc, in_maps, list(range(NCORES))).results
    print(f"[kernel5] device compile+run {_time.time()-_t0:.1f}s", flush=True)

    spk2 = np.empty((T, B_FULL, NO), dtype=np.float32)
    mem2 = np.empty((T, B_FULL, NO), dtype=np.float32)
    with ThreadPoolExecutor(8) as ex:
        list(ex.map(lambda c: unpack_core(res[c], spk2, mem2, c),
                    range(NCORES)))
    return spk2, mem2


# revision 54
# speedup vs baseline: 1.1517x; 1.0775x over previous
"""v5: SNN on 8 trn2 cores, data-parallel over batch.

Per core: B_c = 125,000 = NBL(25 lanes) x NCOLS(5000 cols); no padding
(8 * 125,000 = 1,000,000 exactly).

Precision scheme (emulated rel err 2.2e-3 vs 2e-2 budget):
  x = xh(fp16) + xl(fp8e5m2, exactly x - xh to ~2^-3 of the residual)
  L1 psum = w1h16.xh + w1l16.xh + w8e5.xl + ones*(-1/2) + r1.s1_{t-1}
  mem1 = beta*mem1 + psum1 (DVE fp32), s1 = Sign(mem1 - 1) in {-1,+1} fp16
  L2 psum = w2h16.s1 + w2l16.s1 + ones*(sum w2/2 - 1/2) + r2.s2_{t-1}
  mem2 = beta*mem2 + psum2 (fp32) -> out as fp16 (SWDGE cast-DMA)
  s2 = Sign(mem2 - 1) fp8e4 -> out directly (host maps {-1,+1}->{0,1})

IO per core: in 10MB fp16 + 5MB fp8; out 7.5MB fp16 + 3.75MB fp8.
DMA: one transfer per tensor per step (x on SP ring, spk on SP,
mem cast on gpsimd SWDGE). State tiles are full-width; Tile subtile
deps keep the 5 column-group pipelines independent.
"""

import numpy as np
from contextlib import ExitStack
from concurrent.futures import ThreadPoolExecutor

T = 10
NI, NH, NO = 4, 5, 3
BETA = 0.95
B_FULL = 1_000_000
NCORES = 8

NBL = 25
NCOLS = 5000
NPB = 1024
BC = NBL * NCOLS          # 125,000 per core

XR = NBL * NI             # 100
M1 = NBL * NH             # 125
M2 = NBL * NO             # 75


def set_geometry(ncols, npb=1024):
    global NCOLS, NPB, BC
    NCOLS, NPB = ncols, npb
    BC = NBL * NCOLS


def make_weights(w1, w2):
    import ml_dtypes
    e5 = ml_dtypes.float8_e5m2
    e4 = ml_dtypes.float8_e4m3
    f16 = np.float16
    w1 = np.asarray(w1, np.float64)
    w2 = np.asarray(w2, np.float64)
    # all lhsT free dims padded with zero columns to 128 so the compiler's
    # Fast-Weight-Load kicks in (requires NumWeights==128); the extra psum
    # rows compute zeros and are never read
    # W1 [101, 128]: row (bl,i) -> col (bl,h); ones row 100 = -1/2
    w1f = np.zeros((XR + 1, 128), np.float64)
    for bl in range(NBL):
        w1f[4 * bl : 4 * bl + 4, 5 * bl : 5 * bl + 5] = w1.T
    w1f[XR, 0:M1] = -0.5
    w1h = w1f.astype(f16)
    w1l = (w1f - w1h.astype(np.float64)).astype(f16)
    w8 = w1f[0:XR].astype(e5)
    # r1 [125, 128] = -I/2 (fp16 exact)
    r1 = np.zeros((M1, 128), np.float64)
    r1[:, 0:M1] = -0.5 * np.eye(M1)
    r1 = r1.astype(f16)
    # W2 [126, 128]: row (bl,h) -> col (bl,o); ones row = sum w2/2 - 1/2
    w2f = np.zeros((M1 + 1, 128), np.float64)
    a = w2 / 2.0
    for bl in range(NBL):
        w2f[5 * bl : 5 * bl + 5, 3 * bl : 3 * bl + 3] = a.T
        w2f[M1, 3 * bl : 3 * bl + 3] = a.sum(axis=1) - 0.5
    w2h = w2f.astype(f16)
    w2l = (w2f - w2h.astype(np.float64)).astype(f16)
    # t=0 variant: const row without the -1/2 reset term (s2=-1 cancels it)
    w2f0 = w2f.copy()
    w2f0[M1, 0:M2] += 0.5
    w2h0 = w2f0.astype(f16)
    w2l0 = (w2f0 - w2h0.astype(np.float64)).astype(f16)
    # r2 [75, 128] = -I/2 (fp8e4 exact)
    r2 = np.zeros((M2, 128), np.float64)
    r2[:, 0:M2] = -0.5 * np.eye(M2)
    r2 = r2.astype(e4)
    return w1h, w1l, w8, r1, w2h, w2l, w2h0, w2l0, r2


def _split_multi_waits(nc):
    """Walrus codegen accepts only ONE sync-wait per compute instruction.
    Hoist extras onto pure-sync EventSemaphore instructions."""
    import concourse.mybir as mybir

    for f in nc.m.functions:
        for blk in f.blocks:
            out = []
            for ins in blk.instructions:
                si = ins.sync_info
                if (
                    si is not None
                    and len(si.on_wait) > 1
                    and not isinstance(ins, mybir.InstEventSemaphore)
                ):
                    waits = list(si.on_wait)
                    for j, w in enumerate(waits[:-1]):
                        out.append(
                            mybir.InstEventSemaphore(
                                name=f"{ins.name}-ws{j}",
                                engine=ins.engine,
                                ins=[],
                                outs=[],
                                sync_info=mybir.SyncInfo(
                                    on_wait=[w], on_update=[]
                                ),
                            )
                        )
                    ins.sync_info = mybir.SyncInfo(
                        on_wait=[waits[-1]], on_update=list(si.on_update)
                    )
                out.append(ins)
            blk.instructions = out


def build_nc_v5(split_waits=True, repeat=1, skip_in=False, skip_out=False,
                out_mode="sp", npb=None, ps_bufs=2, skip_act=False,
                skip_dve=False, skew=1, stt2_pool=0, fine1=False):
    import concourse.bass as bass
    import concourse.mybir as mybir
    from concourse.tile import TileContext

    f32 = mybir.dt.float32
    f16 = mybir.dt.float16
    e5 = mybir.dt.float8e5
    e4 = mybir.dt.float8e4
    Act = mybir.ActivationFunctionType
    mult = mybir.AluOpType.mult
    add = mybir.AluOpType.add

    groups = []
    c0 = 0
    while c0 < NCOLS:
        n = min(npb or NPB, NCOLS - c0)
        groups.append((c0, n))
        c0 += n

    nc = bass.Bass()
    xh_d = nc.declare_dram_parameter("xh", [T, XR, NCOLS], f16, isOutput=False)
    xl_d = nc.declare_dram_parameter("xl", [T, XR, NCOLS], e5, isOutput=False)
    w1h_d = nc.declare_dram_parameter("w1h", [XR + 1, 128], f16, isOutput=False)
    w1l_d = nc.declare_dram_parameter("w1l", [XR + 1, 128], f16, isOutput=False)
    w8_d = nc.declare_dram_parameter("w8", [XR, 128], e5, isOutput=False)
    r1_d = nc.declare_dram_parameter("r1", [M1, 128], f16, isOutput=False)
    w2h_d = nc.declare_dram_parameter("w2h", [M1 + 1, 128], f16, isOutput=False)
    w2l_d = nc.declare_dram_parameter("w2l", [M1 + 1, 128], f16, isOutput=False)
    w2h0_d = nc.declare_dram_parameter("w2h0", [M1 + 1, 128], f16, isOutput=False)
    w2l0_d = nc.declare_dram_parameter("w2l0", [M1 + 1, 128], f16, isOutput=False)
    r2_d = nc.declare_dram_parameter("r2", [M2, 128], e4, isOutput=False)
    ones_d = nc.declare_dram_parameter("ones", [1, NCOLS], f16, isOutput=False)
    spk_d = nc.declare_dram_parameter("spk2", [T, M2, NCOLS], e4, isOutput=True)
    mem_d = nc.declare_dram_parameter("mem2", [T, M2, NCOLS], f32, isOutput=True)

    with ExitStack() as ctx:
        tc = ctx.enter_context(TileContext(nc))
        wp = ctx.enter_context(tc.tile_pool(name="wp", bufs=1))
        st = ctx.enter_context(tc.tile_pool(name="st", bufs=1))
        xp = ctx.enter_context(tc.tile_pool(name="xp", bufs=1))
        ps = ctx.enter_context(tc.tile_pool(name="ps", bufs=ps_bufs, space="PSUM"))

        negone = wp.tile([128, 1], f32, tag="negone")
        nc.vector.memset(negone[:], -1.0)
        w1h = wp.tile([XR + 1, 128], f16, tag="w1h")
        w1l = wp.tile([XR + 1, 128], f16, tag="w1l")
        w8 = wp.tile([XR, 128], e5, tag="w8")
        r1 = wp.tile([M1, 128], f16, tag="r1")
        w2h = wp.tile([M1 + 1, 128], f16, tag="w2h")
        w2l = wp.tile([M1 + 1, 128], f16, tag="w2l")
        w2h0 = wp.tile([M1 + 1, 128], f16, tag="w2h0")
        w2l0 = wp.tile([M1 + 1, 128], f16, tag="w2l0")
        r2 = wp.tile([M2, 128], e4, tag="r2")
        for tl, dr in ((w1h, w1h_d), (w1l, w1l_d), (w8, w8_d), (r1, r1_d),
                       (w2h, w2h_d), (w2l, w2l_d), (w2h0, w2h0_d),
                       (w2l0, w2l0_d), (r2, r2_d)):
            nc.sync.dma_start(tl[:], dr[:])

        # persistent state (full width; subtile deps pipeline the groups)
        m1t = st.tile([M1, NCOLS], f32, tag="m1t", name="m1t")
        s1t = st.tile([M1 + 1, NCOLS], f16, tag="s1t", name="s1t")
        # layer-2 state ping-pongs by step parity: the output DMA reads one
        # buffer while the next step's update writes the other (kills the
        # per-step WAR stall on DMA completion)
        m2t_pp = [st.tile([M2, NCOLS], f32, tag=f"m2t{p}", name=f"m2t{p}")
                  for p in range(2)]
        s2t_pp = [st.tile([M2, NCOLS], e4, tag=f"s2t{p}", name=f"s2t{p}")
                  for p in range(2)]
        # no state memsets: the t=0 path writes every state tile before any
        # read (constants fold exactly: ones*(-1/2) + r.(-1) == 0)
        nc.sync.dma_start(s1t[M1 : M1 + 1, :], ones_d[:])
        if skip_act:   # timing-ablation only: states init-once
            nc.vector.memset(s1t[0:M1, :], -1.0)
            for p in range(2):
                nc.vector.memset(s2t_pp[p][:], -1.0)
        if skip_dve:
            nc.vector.memset(m1t[:], 0.0)
            for p in range(2):
                nc.vector.memset(m2t_pp[p][:], 0.0)

        # x ring buffers (3-deep): ones row preset once
        xhs = [xp.tile([XR + 1, NCOLS], f16, tag=f"xh_{r}", name=f"xh_{r}")
               for r in range(3)]
        xls = [xp.tile([XR, NCOLS], e5, tag=f"xl_{r}", name=f"xl_{r}")
               for r in range(3)]
        for r in range(3):
            nc.sync.dma_start(xhs[r][XR : XR + 1, :], ones_d[:])
            if skip_in:
                nc.sync.dma_start(xhs[r][0:XR, :], xh_d[r])
                nc.sync.dma_start(xls[r][:, :], xl_d[r])

        def mm(out_ap, w_ap, rhs_ap, start, stop):
            n = out_ap.shape[-1]
            o = 0
            while o < n:
                k = min(512, n - o)
                nc.tensor.matmul(
                    out_ap[:, o : o + k], w_ap, rhs_ap[:, o : o + k],
                    start=start, stop=stop,
                )
                o += k

        for tg in range(repeat * T):
            t = tg % T
            ring = tg % 3
            xh_, xl_ = xhs[ring], xls[ring]
            if not skip_in:
                if tg == 0 and len(groups) > 1:
                    # cold start: land the first group's columns first so
                    # the first matmul isn't gated on the full transfer
                    n0 = groups[0][1]
                    nc.sync.dma_start(xh_[0:XR, 0:n0], xh_d[t, :, 0:n0])
                    nc.sync.dma_start(xl_[:, 0:n0], xl_d[t, :, 0:n0])
                    nc.sync.dma_start(xh_[0:XR, n0:NCOLS], xh_d[t, :, n0:NCOLS])
                    nc.sync.dma_start(xl_[:, n0:NCOLS], xl_d[t, :, n0:NCOLS])
                else:
                    nc.sync.dma_start(xh_[0:XR, :], xh_d[t])
                    nc.sync.dma_start(xl_[:, :], xl_d[t])
            first = tg == 0

            def emit_L1(g, tg=tg, xh_=xh_, xl_=xl_, first=first):
                c0, n = groups[g]
                cs = slice(c0, c0 + n)
                ps1 = ps.tile([128, n], f32, tag="ps1", name=f"ps1_{tg}_{c0}")
                if first:
                    mm(ps1[:, 0:n], w1h[0:XR, :], xh_[0:XR, cs],
                       start=True, stop=False)
                    mm(ps1[:, 0:n], w1l[0:XR, :], xh_[0:XR, cs],
                       start=False, stop=False)
                    mm(ps1[:, 0:n], w8[:], xl_[:, cs], start=False, stop=True)
                    nc.vector.tensor_scalar_add(m1t[:, cs], ps1[0:M1, 0:n], 0.0)
                else:
                    mm(ps1[:, 0:n], w1h[:], xh_[:, cs], start=True, stop=False)
                    mm(ps1[:, 0:n], w1l[:], xh_[:, cs], start=False, stop=False)
                    mm(ps1[:, 0:n], w8[:], xl_[:, cs], start=False, stop=False)
                    mm(ps1[:, 0:n], r1[:], s1t[0:M1, cs],
                       start=False, stop=True)
                    if not skip_dve:
                        if fine1 and n > 512:
                            for o in range(0, n, 512):
                                k = min(512, n - o)
                                os_ = slice(c0 + o, c0 + o + k)
                                nc.vector.scalar_tensor_tensor(
                                    m1t[:, os_], m1t[:, os_], BETA,
                                    ps1[0:M1, o : o + k], mult, add,
                                )
                                nc.scalar.activation(
                                    s1t[0:M1, os_], m1t[:, os_],
                                    Act.Sign, bias=negone[0:M1, :],
                                )
                        else:
                            nc.vector.scalar_tensor_tensor(
                                m1t[:, cs], m1t[:, cs], BETA, ps1[0:M1, 0:n],
                                mult, add,
                            )
                if not skip_act and not (fine1 and n > 512 and not first
                                         and not skip_dve):
                    nc.scalar.activation(
                        s1t[0:M1, cs], ps1[0:M1, 0:n] if skip_dve else m1t[:, cs],
                        Act.Sign, bias=negone[0:M1, :],
                    )
                elif skip_dve:
                    nc.scalar.activation(
                        s1t[0:M1, cs], ps1[0:M1, 0:n], Act.Sign,
                        bias=negone[0:M1, :],
                    )

            def emit_L2(g, tg=tg, first=first):
                c0, n = groups[g]
                cs = slice(c0, c0 + n)
                cur, prv = tg % 2, (tg - 1) % 2
                m2c, m2p = m2t_pp[cur], m2t_pp[prv]
                s2c, s2p = s2t_pp[cur], s2t_pp[prv]
                ps2 = ps.tile([128, n], f32, tag="ps2", name=f"ps2_{tg}_{c0}")
                if first:
                    mm(ps2[:, 0:n], w2h0[:], s1t[:, cs], start=True, stop=False)
                    mm(ps2[:, 0:n], w2l0[:], s1t[:, cs], start=False, stop=True)
                    nc.vector.tensor_scalar_add(m2c[:, cs], ps2[0:M2, 0:n], 0.0)
                else:
                    # r2 first: its operand (s2 of step t-1) is ready before
                    # this step's s1, so PE needn't wait on ACT
                    mm(ps2[:, 0:n], r2[:], s2p[:, cs], start=True, stop=False)
                    mm(ps2[:, 0:n], w2h[:], s1t[:, cs], start=False, stop=False)
                    mm(ps2[:, 0:n], w2l[:], s1t[:, cs], start=False, stop=True)
                    if not skip_dve:
                        use_pool = (stt2_pool == 2
                                    or (stt2_pool == 1 and g % 2 == 1))
                        eng = nc.gpsimd if use_pool else nc.vector
                        eng.scalar_tensor_tensor(
                            m2c[:, cs], m2p[:, cs], BETA, ps2[0:M2, 0:n],
                            mult, add,
                        )
                if not skip_act:
                    nc.scalar.activation(
                        s2c[:, cs], ps2[0:M2, 0:n] if skip_dve else m2c[:, cs],
                        Act.Sign, bias=negone[0:M2, :],
                    )
                elif skip_dve:
                    nc.scalar.activation(
                        s2c[:, cs], ps2[0:M2, 0:n], Act.Sign,
                        bias=negone[0:M2, :],
                    )

            # software-pipelined emission: engines execute their streams in
            # FIFO order, so L2(g) (gated on this group's ACT sign) is
            # emitted `skew` groups behind L1 to avoid head-of-line blocking
            ng = len(groups)
            for slot in range(ng + skew):
                if slot < ng:
                    emit_L1(slot)
                if slot >= skew:
                    emit_L2(slot - skew)
            if not skip_out:
                m2c, s2c = m2t_pp[tg % 2], s2t_pp[tg % 2]
                last = tg == repeat * T - 1
                if last and len(groups) > 1:
                    # tail: per-group transfers start as each group finishes
                    # instead of one serial 1.9MB DMA after the last compute
                    for (c0, n) in groups:
                        cs = slice(c0, c0 + n)
                        nc.sync.dma_start(mem_d[t, :, cs], m2c[:, cs])
                        nc.sync.dma_start(spk_d[t, :, cs], s2c[:, cs])
                elif out_mode == "sp":
                    nc.sync.dma_start(mem_d[t], m2c[:, :])
                    nc.sync.dma_start(spk_d[t], s2c[:, :])
                else:  # "act"
                    nc.scalar.dma_start(mem_d[t], m2c[:, :])
                    nc.scalar.dma_start(spk_d[t], s2c[:, :])

    if split_waits:
        _split_multi_waits(nc)
    return nc


def prep_core_x(xh_full, xl_full, c):
    # [T, BC, NI] -> rows (bl, i) x cols
    sl = slice(c * BC, (c + 1) * BC)
    out = []
    for arr in (xh_full, xl_full):
        v = arr[:, sl, :].reshape(T, NBL, NCOLS, NI)
        v = np.ascontiguousarray(v.transpose(0, 1, 3, 2)).reshape(T, XR, NCOLS)
        out.append(v)
    return out


def unpack_core(res_c, spk2, mem2, c):
    sl = slice(c * BC, (c + 1) * BC)
    s = res_c["spk2"]   # [T, 75, NCOLS] fp8e4 in {-1,+1}
    m = res_c["mem2"]   # [T, 75, NCOLS] fp16
    sv = s.astype(np.float32).reshape(T, NBL, NO, NCOLS).transpose(0, 1, 3, 2)
    mv = m.astype(np.float32).reshape(T, NBL, NO, NCOLS).transpose(0, 1, 3, 2)
    spk2[:, sl, :] = ((sv + 1.0) * 0.5).reshape(T, BC, NO)
    mem2[:, sl, :] = mv.reshape(T, BC, NO)


def prepare_in_maps(x, w1, w2):
    import ml_dtypes
    w1h, w1l, w8, r1, w2h, w2l, w2h0, w2l0, r2 = make_weights(w1, w2)
    xh_full = x.astype(np.float16)
    xl_full = (x - xh_full.astype(np.float32)).astype(ml_dtypes.float8_e5m2)
    with ThreadPoolExecutor(8) as ex:
        xs = list(ex.map(lambda c: prep_core_x(xh_full, xl_full, c),
                         range(NCORES)))
    onesv = np.ones((1, NCOLS), np.float16)
    return [
        {"xh": xs[c][0], "xl": xs[c][1], "w1h": w1h, "w1l": w1l, "w8": w8,
         "r1": r1, "w2h": w2h, "w2l": w2l, "w2h0": w2h0, "w2l0": w2l0,
         "r2": r2, "ones": onesv}
        for c in range(NCORES)
    ]


def kernel(**inputs):
    x = np.asarray(inputs["x"], dtype=np.float32)
    w1 = np.asarray(inputs["w1"], dtype=np.float32)
    w2 = np.asarray(inputs["w2"], dtype=np.float32)

    from concourse.bass_utils import run_bass_kernel_spmd

    nc = build_nc_v5()
    in_maps = prepare_in_maps(x, w1, w2)

    import time as _time
    _t0 = _time.time()
    res = run_bass_kernel_spmd(nc, in_maps, list(range(NCORES))).results
    print(f"[kernel5] device compile+run {_time.time()-_t0:.1f}s", flush=True)

    spk2 = np.empty((T, B_FULL, NO), dtype=np.float32)
    mem2 = np.empty((T, B_FULL, NO), dtype=np.float32)
    with ThreadPoolExecutor(8) as ex:
        list(ex.map(lambda c: unpack_core(res[c], spk2, mem2, c),
                    range(NCORES)))
    return spk2, mem2
